# revision 30
# baseline (speedup 1.0000x reference)
"""Fused multi-head tanh-attention kernel for Trainium2 (8 NeuronCores).

Problem: y[s,b,:] = concat_h( softmax_t(tanh(q_h k_h^T / 8) - 10000*(1-mask)) @ v_h )
with q/k/v = per-head projections of x.  Shapes: x [1024,16,512], mask [16,1024],
w* [8,64,512] -> y [1024,16,512].

Strategy: batch-parallel over 8 cores (2 batches per core).  Per core, a fully
fused flash-style pipeline keeps the [S,S] score matrices in PSUM/SBUF only:
  - masked keys are COMPACTED away: the host computes, per batch, the index
    list of valid keys (mask=1) and the kernel gathers those x rows via
    indirect DMA, so the key dimension shrinks from 8 to ceil(nb/128) (=7
    here) 128-chunks.  Padding slots carry a compacted mask of 0 and are
    zeroed exactly like reference's exp(-10000) == 0 underflow,
  - x and the gathered x are transposed on-chip via PE-transpose (contraction
    dim on partitions); all matmuls run in float32r (full PE rate, ~1e-4 rel
    err),
  - scores are built in scoresT [t,s] layout; the compacted mask is folded
    into the v tiles (v rows and the appended ones-columns are scaled by it),
  - tanh+exp run on the scalar engine (one table set holds both; exp covers
    two t-chunks per instruction), PV accumulates unnormalized out^T plus the
    softmax denominator (ones-column trick), which is PE-transposed back and
    divided on the vector engine,
  - the scalar engine is the bottleneck, so all other work (projections, v
    construction, gathers, the previous head's epilogue, weight transposes)
    is emitted interleaved into the attention t-loop as background tasks so
    the in-order engine queues never starve the activation engine.
"""

import sys

sys.path.insert(0, "/opt/trn_rl_repo")

from contextlib import ExitStack

import numpy as np

S, B, D, H, DH = 1024, 16, 512, 8, 64
NCORES = 8
BPC = B // NCORES  # batches per core
SC = S // 128  # 8 query chunks
DC = D // 128  # 4 d-chunks
DEFAULT_NKC = 7  # key chunks after mask compaction (ceil(max_nb/128))

_compiled = {}


def _nsplits(total):
    out, p = [], 0
    while p < total:
        sz = min(512, total - p)
        out.append((p, sz))
        p += sz
    return out


def _groups(nkc):
    gs, i = [], 0
    while i < nkc:
        gs.append(tuple(range(i, min(i + 2, nkc))))
        i += 2
    return gs


def _make_pools(tc, ctx):
    pools = {}
    pools["singles"] = ctx.enter_context(tc.tile_pool(name="singles", bufs=1))
    pools["nat"] = ctx.enter_context(tc.tile_pool(name="nat", bufs=3))
    pools["qk"] = ctx.enter_context(tc.tile_pool(name="qk", bufs=4))
    pools["vh"] = ctx.enter_context(tc.tile_pool(name="vh", bufs=8))
    pools["tanh"] = ctx.enter_context(tc.tile_pool(name="tanh", bufs=3))
    pools["exp"] = ctx.enter_context(tc.tile_pool(name="exp", bufs=3))
    pools["outT"] = ctx.enter_context(tc.tile_pool(name="outT", bufs=2))
    pools["outsb"] = ctx.enter_context(tc.tile_pool(name="outsb", bufs=2))
    pools["small"] = ctx.enter_context(tc.tile_pool(name="small", bufs=4))
    # PSUM: 8 banks.  ps_big ([128,1024]f32 slots = 2 banks, bufs=3 = 6
    # banks) rotates scores / projections / v chunks / all transposes.
    # ps_o (2 banks, bufs=1) holds the per-head PV accumulator.
    pools["ps_big"] = ctx.enter_context(
        tc.tile_pool(name="ps_big", bufs=3, space="PSUM")
    )
    pools["ps_o"] = ctx.enter_context(tc.tile_pool(name="ps_o", bufs=1, space="PSUM"))
    return pools


def _emit(nc, tc, pools, tile, mybir, bass, aps, nkc, u=0):
    f32 = mybir.dt.float32
    f32r = mybir.dt.float32r
    i32 = mybir.dt.int32
    AF = mybir.ActivationFunctionType
    Alu = mybir.AluOpType
    x_d, kidx_d, kmsk_d, wq_d, wk_d, wv_d, id_d, y_d = aps
    NK = nkc * 128

    singles = pools["singles"]
    nat = pools["nat"]
    qk_pool = pools["qk"]
    vh_pool = pools["vh"]
    tanh_pool = pools["tanh"]
    exp_pool = pools["exp"]
    outT_pool = pools["outT"]
    outsb_pool = pools["outsb"]
    small = pools["small"]
    ps_big = pools["ps_big"]
    ps_o = pools["ps_o"]

    yr = y_d.rearrange("(c p) b e -> p c b e", p=128)
    xflat = x_d.rearrange("s b d -> (s b) d")

    # ---------------- prologue ------------------------------------------
    ident = singles.tile([128, 128], f32r, tag="ident", name=f"ident_u{u}")
    nc.sync.dma_start(ident, id_d)
    fill64 = singles.tile([128, nkc, 64], f32, tag="fill64", name=f"fill64_u{u}")
    nc.vector.memset(fill64, 1.0)

    # compacted key indices / mask columns, per batch
    kidx = {}
    kmsk = {}
    for b in range(BPC):
        ki = singles.tile([128, nkc], i32, tag=f"kidx{b}", name=f"kidx{b}_u{u}")
        nc.sync.dma_start(ki, kidx_d[b].rearrange("(c p) -> p c", p=128))
        kidx[b] = ki
        km = small.tile([128, nkc], f32, tag="msk", name=f"kmsk{b}_u{u}")
        nc.sync.dma_start(km, kmsk_d[b].rearrange("(c p) -> p c", p=128))
        kmsk[b] = km
        mf = singles.tile([128, nkc, 64], f32, tag=f"mfill{b}", name=f"mfill{b}_u{u}")
        for tck in range(nkc):
            nc.vector.tensor_scalar(
                mf[:, tck, :], fill64[:, tck, :], km[:, tck : tck + 1], None, Alu.mult
            )
        kmsk[b, "fill"] = mf

    # ---- emitters ------------------------------------------------------
    wTq = {}
    wTk = {}
    wTv = {}

    def emit_wqk_tr(nm, w_d, wT, hp):
        w_nat = nat.tile([128, D], f32r, tag="nat", name=f"w_nat_u{u}")
        nc.sync.dma_start(w_nat, w_d[2 * hp : 2 * hp + 2].rearrange("h e d -> (h e) d"))
        wt = singles.tile(
            [128, DC, 128], f32r, tag=f"wT{nm}{hp}", name=f"wT{nm}{hp}_u{u}"
        )
        wT[hp] = wt
        for dc in range(DC):
            pst = ps_big.tile([128, 128], f32r, tag="ps_big", name=f"pstr_u{u}")
            nc.tensor.transpose(pst, w_nat[:, dc * 128 : dc * 128 + 128], ident)
            nc.vector.tensor_copy(wt[:, dc, :], pst)

    def emit_wv_tr(q4):
        wt = singles.tile([128, DC, 256], f32r, tag=f"wTv{q4}", name=f"wTv{q4}_u{u}")
        wTv[q4] = wt
        for half in range(2):
            w_nat = nat.tile([128, D], f32r, tag="nat", name=f"w_nat_u{u}")
            h0 = 4 * q4 + 2 * half
            nc.sync.dma_start(w_nat, wv_d[h0 : h0 + 2].rearrange("h e d -> (h e) d"))
            for dc in range(DC):
                pst = ps_big.tile([128, 128], f32r, tag="ps_big", name=f"pstr_u{u}")
                nc.tensor.transpose(pst, w_nat[:, dc * 128 : dc * 128 + 128], ident)
                nc.vector.tensor_copy(wt[:, dc, half * 128 : half * 128 + 128], pst)

    xbT = {}  # full x^T (query side)
    xkT = {}  # gathered x^T (key side)

    def alloc_xbT(b):
        for dc in range(DC):
            xbT[b, dc] = singles.tile(
                [128, S], f32r, tag=f"xbT{b}{dc}", name=f"xbT{b}{dc}_u{u}"
            )

    def alloc_xkT(b):
        for dc in range(DC):
            xkT[b, dc] = singles.tile(
                [128, NK], f32r, tag=f"xkT{b}{dc}", name=f"xkT{b}{dc}_u{u}"
            )

    def emit_x_tr(b, sc):
        x_nat = nat.tile([128, D], f32r, tag="nat", name=f"x_nat_u{u}")
        nc.sync.dma_start(x_nat, x_d[sc * 128 : sc * 128 + 128, b, :])
        for dc in range(DC):
            pst = ps_big.tile([128, 128], f32r, tag="ps_big", name=f"pstr_u{u}")
            nc.tensor.transpose(pst, x_nat[:, dc * 128 : dc * 128 + 128], ident)
            nc.vector.tensor_copy(xbT[b, dc][:, sc * 128 : sc * 128 + 128], pst)

    def emit_xk_dma(b, c):
        xk_nat = nat.tile([128, D], f32r, tag="xknat", name=f"xk_nat_u{u}")
        nc.gpsimd.indirect_dma_start(
            out=xk_nat,
            out_offset=None,
            in_=xflat,
            in_offset=bass.IndirectOffsetOnAxis(ap=kidx[b][:, c : c + 1], axis=0),
        )
        return xk_nat

    def emit_xk_tr(b, c, xk_nat):
        for dc in range(DC):
            pst = ps_big.tile([128, 128], f32r, tag="ps_big", name=f"pstr_u{u}")
            nc.tensor.transpose(pst, xk_nat[:, dc * 128 : dc * 128 + 128], ident)
            nc.vector.tensor_copy(xkT[b, dc][:, c * 128 : c * 128 + 128], pst)

    def emit_xk_gather(b, c):
        emit_xk_tr(b, c, emit_xk_dma(b, c))

    qkT = {}

    def emit_proj_q(b, hp):
        psp = ps_big.tile([128, S], f32, tag="ps_big", name=f"psp_u{u}")
        for dc in range(DC):
            for off, sz in _nsplits(S):
                nc.tensor.matmul(
                    psp[:, off : off + sz],
                    wTq[hp][:, dc, :],
                    xbT[b, dc][:, off : off + sz],
                    start=(dc == 0),
                    stop=(dc == DC - 1),
                )
        t = qk_pool.tile([128, S], f32r, tag="qkT", name=f"qkTq_u{u}")
        qkT[b, hp, "q"] = t
        nc.vector.tensor_copy(t, psp)

    def emit_proj_k(b, hp):
        psp = ps_big.tile([128, S], f32, tag="ps_big", name=f"psp_u{u}")
        for dc in range(DC):
            for off, sz in _nsplits(NK):
                nc.tensor.matmul(
                    psp[:, off : off + sz],
                    wTk[hp][:, dc, :],
                    xkT[b, dc][:, off : off + sz],
                    start=(dc == 0),
                    stop=(dc == DC - 1),
                )
        t = qk_pool.tile([128, S], f32r, tag="qkT", name=f"qkTk_u{u}")
        qkT[b, hp, "k"] = t
        nc.vector.tensor_copy(t[:, 0:NK], psp[:, 0:NK])

    vh = {}

    def alloc_vh(b, q4):
        for h in range(4 * q4, 4 * q4 + 4):
            vh[b, h] = vh_pool.tile(
                [128, nkc, 128], f32r, tag="vh", name=f"vh{b}_{h}_u{u}"
            )
            nc.vector.tensor_copy(vh[b, h][:, :, 64:128], kmsk[b, "fill"])

    def emit_v_chunk(b, q4, tck):
        psv = ps_big.tile([128, 256], f32, tag="ps_big", name=f"psv_u{u}")
        for dc in range(DC):
            nc.tensor.matmul(
                psv,
                xkT[b, dc][:, tck * 128 : tck * 128 + 128],
                wTv[q4][:, dc, :],
                start=(dc == 0),
                stop=(dc == DC - 1),
            )
        for h_in, h in enumerate(range(4 * q4, 4 * q4 + 4)):
            nc.vector.tensor_scalar(
                vh[b, h][:, tck, 0:64],
                psv[:, h_in * 64 : h_in * 64 + 64],
                kmsk[b][:, tck : tck + 1],
                None,
                Alu.mult,
            )

    def out_stage_parts(b, h, pso):
        state = {}

        def p1():
            outT = outT_pool.tile([128, S], f32r, tag="outT", name=f"outT_u{u}")
            nc.vector.tensor_copy(outT, pso)
            state["outT"] = outT

        def p2():
            pst = ps_big.tile([128, SC, 128], f32r, tag="ps_big", name=f"psto_u{u}")
            for sc in range(SC):
                nc.tensor.transpose(
                    pst[:, sc, 0:128],
                    state["outT"][:, sc * 128 : sc * 128 + 128],
                    ident,
                )
            state["pst"] = pst

        def p3():
            pst = state["pst"]
            rec = small.tile([128, SC], f32, tag="rec", name=f"rec_u{u}")
            nc.vector.reciprocal(rec, pst[:, :, 64])
            osb = outsb_pool.tile([128, SC, 64], f32, tag="osb", name=f"osb_u{u}")
            for sc in range(SC):
                nc.vector.tensor_scalar(
                    osb[:, sc, :], pst[:, sc, 0:64], rec[:, sc : sc + 1], None, Alu.mult
                )
            nc.sync.dma_start(yr[:, :, b, h * 64 : h * 64 + 64], osb)

        return [p1, p2, p3]

    # ---------------- bootstrap -----------------------------------------
    alloc_xbT(0)
    alloc_xkT(0)
    # q-projection chain first: it gates the first QK (needs ALL of x^T),
    # so nothing may sit ahead of it in the PE stream or the DMA queues.
    emit_wqk_tr("q", wq_d, wTq, 0)
    for sc in range(SC):
        emit_x_tr(0, sc)
    emit_proj_q(0, 0)
    xk_nats = [emit_xk_dma(0, c) for c in range(nkc)]
    emit_wqk_tr("k", wk_d, wTk, 0)
    for c in range(nkc):
        emit_xk_tr(0, c, xk_nats[c])
    emit_proj_k(0, 0)
    emit_wv_tr(0)
    alloc_vh(0, 0)
    for tck in range(4):
        emit_v_chunk(0, 0, tck)

    # background task lists per head index
    NH = BPC * H
    bg = {i: [] for i in range(NH + 1)}
    bg[0] += [(lambda tck=tck: emit_v_chunk(0, 0, tck)) for tck in range(4, nkc)]
    bg[0] += [
        (lambda hp=hp: emit_wqk_tr("q", wq_d, wTq, hp)) for hp in range(1, H // 2)
    ]
    bg[1] += [
        (lambda hp=hp: emit_wqk_tr("k", wk_d, wTk, hp)) for hp in range(1, H // 2)
    ]
    bg[1] += [lambda: emit_wv_tr(1)]
    for b in range(BPC):
        base = b * H
        for h in range(1, H, 2):
            if h < H - 1:
                hp = (h + 1) // 2
                bg[base + h] += [
                    lambda b=b, hp=hp: emit_proj_q(b, hp),
                    lambda b=b, hp=hp: emit_proj_k(b, hp),
                ]
        bg[base + 2] += [lambda b=b: alloc_vh(b, 1)]
        bg[base + 2] += [
            (lambda b=b, tck=tck: emit_v_chunk(b, 1, tck)) for tck in range(0, 4)
        ]
        bg[base + 3] += [
            (lambda b=b, tck=tck: emit_v_chunk(b, 1, tck)) for tck in range(4, nkc)
        ]
    if BPC > 1:
        bg[4] += [lambda: alloc_xbT(1), lambda: alloc_xkT(1)]
        bg[4] += [(lambda sc=sc: emit_x_tr(1, sc)) for sc in range(0, 3)]
        bg[5] += [(lambda sc=sc: emit_x_tr(1, sc)) for sc in range(3, 6)]
        bg[5] += [(lambda c=c: emit_xk_gather(1, c)) for c in range(0, 3)]
        bg[6] += [(lambda sc=sc: emit_x_tr(1, sc)) for sc in range(6, SC)]
        bg[6] += [(lambda c=c: emit_xk_gather(1, c)) for c in range(3, nkc)]
        bg[7] += [
            lambda: emit_proj_q(1, 0),
            lambda: emit_proj_k(1, 0),
            lambda: alloc_vh(1, 0),
        ]
        bg[7] += [(lambda tck=tck: emit_v_chunk(1, 0, tck)) for tck in range(nkc)]

    # ---------------- main attention loop --------------------------------
    heads = [(b, h) for b in range(BPC) for h in range(H)]
    groups = _groups(nkc)

    def emit_qk_chunk(b, hp, h2, tck):
        r0 = h2 * 64
        kT = qkT[b, hp, "k"]
        qT = qkT[b, hp, "q"]
        pss = ps_big.tile([128, S], f32, tag="ps_big", name=f"pss_u{u}")
        for sh in range(2):
            nc.tensor.matmul(
                pss[:, sh * 512 : sh * 512 + 512],
                kT[r0 : r0 + 64, tck * 128 : tck * 128 + 128],
                qT[r0 : r0 + 64, sh * 512 : sh * 512 + 512],
                start=True,
                stop=True,
            )
        return pss

    # flat schedule of (head-index, group) so each group's first QK can be
    # emitted one group early (before the previous group's PVs), keeping the
    # activation engine from waiting on the in-order PE queue.
    sched = []
    for hi in range(len(heads)):
        for gi, grp in enumerate(groups):
            sched.append((hi, gi, grp))
    pending_qk = None
    pso = None
    for si, (hi, gi, grp) in enumerate(sched):
        b, h = heads[hi]
        hp, h2 = h // 2, h % 2
        if gi == 0:
            pso = ps_o.tile([128, S], f32, tag="ps_o", name=f"pso_u{u}")
            pso_by_head = getattr(emit_qk_chunk, "_pso", {})
            pso_by_head[hi] = pso
            emit_qk_chunk._pso = pso_by_head
            tasks = list(bg[hi])
            done = 0
        tnh = tanh_pool.tile([128, len(grp), S], f32, tag="tanh", name=f"tnh_u{u}")
        for j, tck in enumerate(grp):
            if j == 0 and pending_qk is not None:
                pss = pending_qk
                pending_qk = None
            else:
                pss = emit_qk_chunk(b, hp, h2, tck)
            nc.scalar.activation(tnh[:, j, :], pss, AF.Tanh, scale=0.125)
        ex = exp_pool.tile([128, len(grp), S], f32r, tag="exp", name=f"ex_u{u}")
        nc.scalar.activation(ex, tnh.rearrange("p a s -> p (a s)"), AF.Exp)
        # drain background work (keeps PE/DVE busy while ACT runs)
        target = (len(tasks) * (gi + 1) + len(groups) - 1) // len(groups)
        while done < target:
            tasks[done]()
            done += 1
        # emit the NEXT group's first QK before this group's PVs
        if si + 1 < len(sched):
            nhi, ngi, ngrp = sched[si + 1]
            nb_, nh_ = heads[nhi]
            pending_qk = emit_qk_chunk(nb_, nh_ // 2, nh_ % 2, ngrp[0])
        for j, tck in enumerate(grp):
            for sh in range(2):
                nc.tensor.matmul(
                    pso[:, sh * 512 : sh * 512 + 512],
                    vh[b, h][:, tck, :],
                    ex[:, j, sh * 512 : sh * 512 + 512],
                    start=(tck == 0),
                    stop=(tck == nkc - 1),
                )
        if gi == len(groups) - 1:
            bg[hi + 1] = out_stage_parts(b, h, pso) + bg[hi + 1]
    for t in bg[NH]:
        t()


def _build(unroll=1, nkc=DEFAULT_NKC):
    import concourse.bass as bass
    import concourse.tile as tile
    from concourse import bacc, mybir

    f32 = mybir.dt.float32
    f32r = mybir.dt.float32r
    i32 = mybir.dt.int32
    NK = nkc * 128
    nc = bacc.Bacc("TRN2", target_bir_lowering=False, debug=False)
    x_d = nc.dram_tensor("x", [S, BPC, D], f32r, kind="ExternalInput").ap()
    kidx_d = nc.dram_tensor("kidx", [BPC, NK], i32, kind="ExternalInput").ap()
    kmsk_d = nc.dram_tensor("kmsk", [BPC, NK], f32, kind="ExternalInput").ap()
    wq_d = nc.dram_tensor("wq", [H, DH, D], f32r, kind="ExternalInput").ap()
    wk_d = nc.dram_tensor("wk", [H, DH, D], f32r, kind="ExternalInput").ap()
    wv_d = nc.dram_tensor("wv", [H, DH, D], f32r, kind="ExternalInput").ap()
    id_d = nc.dram_tensor("ident", [128, 128], f32r, kind="ExternalInput").ap()
    y_d = nc.dram_tensor("y", [S, BPC, D], f32, kind="ExternalOutput").ap()
    with tile.TileContext(nc) as tc, ExitStack() as ctx:
        pools = _make_pools(tc, ctx)
        aps = (x_d, kidx_d, kmsk_d, wq_d, wk_d, wv_d, id_d, y_d)
        for u in range(unroll):
            _emit(nc, tc, pools, tile, mybir, bass, aps, nkc, u)
    nc.compile()
    return nc


def get_compiled(nkc=DEFAULT_NKC):
    if nkc not in _compiled:
        _compiled[nkc] = _build(nkc=nkc)
    return _compiled[nkc]


def _compute_nkc(mask):
    nb_max = int((np.asarray(mask) != 0).sum(axis=1).max())
    return max(1, -(-nb_max // 128))


def make_in_maps(x, mask, wq, wk, wv, nkc=DEFAULT_NKC):
    x = np.asarray(x, np.float32)
    mask = np.asarray(mask, np.float32)
    wq = np.ascontiguousarray(np.asarray(wq, np.float32))
    wk = np.ascontiguousarray(np.asarray(wk, np.float32))
    wv = np.ascontiguousarray(np.asarray(wv, np.float32))
    ident = np.eye(128, dtype=np.float32)
    NK = nkc * 128
    maps = []
    for c in range(NCORES):
        mb = mask[c * BPC : (c + 1) * BPC, :]
        kidx = np.zeros((BPC, NK), np.int32)
        kmsk = np.zeros((BPC, NK), np.float32)
        for b in range(BPC):
            valid = np.nonzero(mb[b] != 0)[0][:NK]
            # row index into the per-core x flattened as [(s b), d]
            kidx[b, : len(valid)] = valid.astype(np.int32) * BPC + b
            kmsk[b, : len(valid)] = 1.0
        maps.append(
            {
                "x": np.ascontiguousarray(x[:, c * BPC : (c + 1) * BPC, :]),
                "kidx": kidx,
                "kmsk": kmsk,
                "wq": wq,
                "wk": wk,
                "wv": wv,
                "ident": ident,
            }
        )
    return maps


def kernel(x, mask, wq, wk, wv):
    from concourse.bass_utils import run_bass_kernel_spmd

    nkc = _compute_nkc(mask)
    nc = get_compiled(nkc)
    in_maps = make_in_maps(x, mask, wq, wk, wv, nkc=nkc)
    res = run_bass_kernel_spmd(nc, in_maps, list(range(NCORES))).results
    y = np.concatenate([r["y"] for r in res], axis=1)
    return np.ascontiguousarray(y.astype(np.float32, copy=False))
